# revision 2
# baseline (speedup 1.0000x reference)
"""Trainium2 Bass kernel for agent-attention (AAGA): 8-core data-parallel over batch.

Math (per batch b):
  qkv = x @ W_qkv + b_qkv ; q,k,v = split(qkv)
  ag  = agent @ W_agent + b_agent ; q_agent,k_agent = split(ag)
  attn1 = softmax(q_agent @ k^T * s)        # [K, N]
  va    = (attn1 @ v) @ W_fc1 + b_fc1       # [K, d]
  attn2 = softmax(q @ k_agent^T * s)        # [N, K]
  out   = (attn2 @ va) @ W_fc2 + b_fc2 + x  # [N, d]

Host-side algebraic folds (everything not involving x is an input):
  q_agent/k_agent computed on host; q,k,v never materialized on device.
  S1^T = x @ (W_k@q_agent^T): b_k drops out of the softmax (shift invariance).
  va-chain: attn1 rows sum to 1, so all later biases fold into a single
       constant row bbig = (b_v@W_fc1+b_fc1)@W_fc2 + b_fc2 ADDED ON HOST.
  Wbig = W_v@W_fc1@W_fc2. Device:
    expS1[t,k] = exp(s*S1 - ln16)  (token-major, fp8)
    avxT[d,k]  = sum_t x[t,d]*expS1[t,k]   computed DIRECTLY transposed via
                 lhsT=xe-tile, rhs=expS1-tile DR matmuls (no PE transposes),
                 plus s1[k] = sum_t expS1[t,k] via a ones rhs.
    vaF[k,:]   = (1/s1)[k] * (avxT^T @ Wbig)[k,:]
    expS2[k,t] = exp(s*S2 + c2 - ln16)
    y[t,:]     = sum_k expS2[k,t]*vaF[k,:]   (256-col tiles, 2 per PSUM bank)
    yden[t]    = sum_k expS2[k,t]            (separate [128,1] matmuls)
  Host epilogue: out = y/yden + bbig + x  (exact fp32).

Cost-model regime: DMA_ENGINES is a serialized 360GB/s resource (~6.2us of
input, ~2.9us of output); PSUM->SBUF casts can only run on Act/DVE
(cols*0.83ns + ~186ns fixed for Act, cols*1.04ns + ~125ns for DVE), so the
y phase batches 4 token-tiles per cast instruction and alternates engines.
"""

import numpy as np
import ml_dtypes

B, N, D, K = 8, 4096, 256, 64
P = 128
NT = N // P        # 32 token tiles
DS = D // P        # 2 contraction subtiles
W = 512            # free-dim chunk for S2
NC2 = N // W       # 8 chunks

# input streaming chunks (in token tiles)
XT_CHUNKS = [4, 8, 10, 10]     # HWDGE (sync)
XE_CHUNKS = [8, 12, 12]        # SWDGE (gpsimd)
# S1 slabs (in token tiles); small last slab shortens the post-DMA tail
SLABS = [4, 8, 8, 8, 4]
# y-phase groups (in token tiles): 4-tile groups (2 PSUM banks, 2x256 cols
# per bank); small final groups cut the output-DMA tail
YGROUPS = [4, 4, 4, 4, 4, 4, 4, 2, 2]
# output DMA batching (in tiles per DMA)
YDMA = [8, 8, 8, 4, 2, 2]

_BF16 = ml_dtypes.bfloat16
_FP8 = ml_dtypes.float8_e4m3

_CACHE = {}


def _build_nc():
    import concourse.bass as bass
    import concourse.tile as tile
    from concourse import bacc, mybir

    f32 = mybir.dt.float32
    bf16 = mybir.dt.bfloat16
    fp8 = mybir.dt.float8e4
    Exp = mybir.ActivationFunctionType.Exp
    DR = mybir.MatmulPerfMode.DoubleRow
    Copy = mybir.ActivationFunctionType.Copy
    ts = bass.ts

    nc = bacc.Bacc("TRN2", target_bir_lowering=False, debug=False)

    xT_d = nc.declare_dram_parameter("xT", [P, DS, N], fp8, isOutput=False)
    xe_d = nc.declare_dram_parameter("xe", [P, NT, D], fp8, isOutput=False)
    wkq8_d = nc.declare_dram_parameter("wkq8", [P, DS, 2 * K], fp8, isOutput=False)
    wcombo_d = nc.declare_dram_parameter("wcombo", [P, DS, D], bf16, isOutput=False)
    fc_d = nc.declare_dram_parameter("fc", [K, 1], bf16, isOutput=False)
    ye_d = nc.declare_dram_parameter("ye", [P, NT, D], fp8, isOutput=True)
    yden_d = nc.declare_dram_parameter("yden", [P, NT], bf16, isOutput=True)

    with tile.TileContext(nc) as tc:
        with (
            tc.tile_pool(name="sb", bufs=1) as sb,
            tc.tile_pool(name="yout", bufs=4) as yout,
        ):
            # ---------------- input DMAs ----------------
            wkq8 = sb.tile([P, DS, 2 * K], fp8)
            nc.gpsimd.dma_start(out=wkq8, in_=wkq8_d[:, :, :])
            wk = wkq8[:, :, 0:K]
            wq = wkq8[:, :, K : 2 * K]

            xT = sb.tile([P, DS, N], fp8)
            xe = sb.tile([P, NT, D], fp8)
            t0 = 0
            for ct in XT_CHUNKS:
                nc.sync.dma_start(
                    out=xT[:, :, P * t0 : P * (t0 + ct)],
                    in_=xT_d[:, :, P * t0 : P * (t0 + ct)],
                )
                t0 += ct
            fc = sb.tile([K, 1], bf16)
            nc.sync.dma_start(out=fc, in_=fc_d[:, :])
            wcombo = sb.tile([P, DS, D], bf16)
            nc.sync.dma_start(out=wcombo, in_=wcombo_d[:, :, :])
            t0 = 0
            for ct in XE_CHUNKS:
                nc.gpsimd.dma_start(
                    out=xe[:, t0 : t0 + ct, :], in_=xe_d[:, t0 : t0 + ct, :]
                )
                t0 += ct
            bias2 = fc[:, 0:1]                      # c2*scale - ln(16)

            expS1 = sb.tile([P, NT, K], fp8)    # token-major exp(S1), /16-shifted
            sh1 = sb.tile([P, 1], f32)
            nc.vector.memset(sh1, -2.772588722239781)   # -ln(16): keeps exp < 240 (fp8 max)
            ones8 = sb.tile([P, 2, 1], fp8)
            nc.vector.memset(ones8, 1.0)
            onesb = sb.tile([K, 1], bf16)
            nc.vector.memset(onesb, 1.0)
            # dummy exp: pulls the 1.3us LoadActFuncSet into the DMA head
            warm = sb.tile([P, 1], f32)
            nc.scalar.activation(warm, sh1, Exp)
            expS2 = sb.tile([K, NC2, W], bf16)          # agent-major exp(S2)

            vaF = sb.tile([K, D], bf16)
            avxT_sb = sb.tile([P, DS, K], bf16)
            rec1 = sb.tile([K, 1], f32)

            # ---- phase 1: S1 slabs (+ streamed avxT accum) then S2 chunks ----
            with (
                tc.tile_pool(name="s1p", bufs=3, space="PSUM") as s1p,  # 3 banks
                tc.tile_pool(name="s2p", bufs=2, space="PSUM") as s2p,  # 4 banks
                tc.tile_pool(name="pX", bufs=1, space="PSUM") as pX,    # 1 bank
            ):
                # avxT accumulator [P, DS, K] plus s1 sums in one bank
                px = pX.tile([P, DS, K + 2], f32, tag="pX")
                den = px[0:K, 0, K : K + 1]             # [K, 1]
                nslab = len(SLABS)
                sstart = [sum(SLABS[:i]) for i in range(nslab)]

                def s1_slab(b):
                    t0, sl = sstart[b], SLABS[b]
                    ps = s1p.tile([P, sl, K], f32, tag="s1p")
                    for j in range(sl):
                        t = t0 + j
                        # DoubleRow: 2 fp8 weights/cell -> 256-contraction in one mm
                        nc.tensor.matmul(
                            ps[:, j, :], xT[:, :, ts(t, P)], wk,
                            start=True, stop=True, perf_mode=DR,
                        )
                    nc.scalar.activation(
                        expS1[:, t0 : t0 + sl, :], ps, Exp,
                        scale=float(D ** -0.5), bias=sh1,
                    )
                    # streamed avxT accumulation: avxT[d,k] += x[t,d]*e1[t,k]
                    for j in range(sl // 2):
                        u = t0 // 2 + j
                        st, sp = (u == 0), (u == NT // 2 - 1)
                        e1u = expS1[:, 2 * u : 2 * u + 2, :]
                        for s in range(DS):
                            nc.tensor.matmul(
                                px[:, s, 0:K],
                                xe[:, 2 * u : 2 * u + 2, ts(s, P)],
                                e1u, start=st, stop=sp, perf_mode=DR,
                            )
                        nc.tensor.matmul(
                            den, e1u, ones8, start=st, stop=sp, perf_mode=DR,
                        )

                def s2_pair(h):
                    # two 512-token chunks in two PSUM banks -> one exp
                    p2 = s2p.tile([K, 2, W], f32, tag="s2p")
                    for g in range(2):
                        c = 2 * h + g
                        nc.tensor.matmul(
                            p2[:, g, :], wq, xT[:, :, ts(c, W)],
                            start=True, stop=True, perf_mode=DR,
                        )
                    nc.scalar.activation(
                        expS2[:, 2 * h : 2 * h + 2, :], p2, Exp,
                        scale=float(D ** -0.5), bias=bias2,
                    )

                for b in range(nslab):
                    s1_slab(b)
                # S2 logits only feed the y phase; running them after the S1
                # stream keeps Act free so expS1 (which gates avxT) never
                # queues behind a 1us S2 exp.
                for h in range(NC2 // 2):
                    s2_pair(h)

                # ---- vaF = rec1 * (avxT^T @ Wbig) ----
                nc.vector.reciprocal(rec1, den)
                nc.vector.tensor_copy(avxT_sb, px[:, :, 0:K])
                vf_ps = s1p.tile([K, D], f32, tag="s1p")
                for s in range(DS):
                    nc.tensor.matmul(
                        vf_ps, avxT_sb[:, s, :], wcombo[:, s, :],
                        start=(s == 0), stop=(s == DS - 1),
                    )
                nc.vector.tensor_scalar_mul(vaF, vf_ps, rec1)

            # ---- y[t,:] = sum_k expS2[k,t]*vaF[k,:]; yden[t] = sum_k expS2 ----
            # 4-tile groups: 2 PSUM banks, two 256-col tiles per bank. Casts
            # batched per group, alternating Act/DVE. Denominators go to a
            # dedicated bank via [128,1] matmuls, copied once at the end.
            with (
                tc.tile_pool(name="ypool", bufs=3, space="PSUM") as ypool,
                tc.tile_pool(name="ydp", bufs=1, space="PSUM") as ydp,
            ):
                yden_ps = ydp.tile([P, NT], f32, tag="ydp")
                g0 = 0
                y_sb = None
                dma_i = 0
                dma_fill = 0
                for gi, gsz in enumerate(YGROUPS):
                    yp = ypool.tile([P, 2, W], f32, tag="ypool")
                    if dma_fill == 0:
                        ysz = YDMA[dma_i]
                        y_sb = yout.tile([P, ysz, D], fp8, tag="ysb")
                        ysb0 = g0
                    for j in range(gsz):
                        t = g0 + j
                        e2t = expS2[:, t // 4, ts(t % 4, P)]
                        nc.tensor.matmul(
                            yp[:, j // 2, (j % 2) * D : (j % 2) * D + D],
                            e2t, vaF, start=True, stop=True,
                        )
                        nc.tensor.matmul(
                            yden_ps[:, t : t + 1], e2t, onesb,
                            start=True, stop=True,
                        )
                    dst = y_sb[:, g0 - ysb0 : g0 - ysb0 + gsz, :]
                    src = yp[:, 0 : (gsz + 1) // 2, 0 : min(gsz, 2) * D]
                    if gi % 2 == 0:
                        nc.scalar.activation(dst, src, Copy)
                    else:
                        nc.vector.tensor_copy(dst, src)
                    dma_fill += gsz
                    if dma_fill == YDMA[dma_i]:
                        nc.sync.dma_start(
                            out=ye_d[:, ysb0 : ysb0 + dma_fill, :],
                            in_=y_sb[:, 0:dma_fill, :],
                        )
                        dma_i += 1
                        dma_fill = 0
                    g0 += gsz
                yden_sb = sb.tile([P, NT], bf16)
                nc.vector.tensor_copy(yden_sb, yden_ps)
                nc.gpsimd.dma_start(out=yden_d[:, :], in_=yden_sb)

    nc.compile()
    return nc


def _get_nc():
    if "nc" not in _CACHE:
        _CACHE["nc"] = _build_nc()
    return _CACHE["nc"]


def _prepare_in_maps(agent, x, W_qkv, b_qkv, W_agent, b_agent, W_fc1, b_fc1, W_fc2, b_fc2):
    # ---- host folds (float64 for stability, cast down at the end) ----
    agent64 = np.asarray(agent, np.float64)
    Wqkv64 = np.asarray(W_qkv, np.float64)
    bqkv64 = np.asarray(b_qkv, np.float64)
    Wag64 = np.asarray(W_agent, np.float64)
    bag64 = np.asarray(b_agent, np.float64)
    Wf1 = np.asarray(W_fc1, np.float64)
    bf1 = np.asarray(b_fc1, np.float64)
    Wf2 = np.asarray(W_fc2, np.float64)
    bf2 = np.asarray(b_fc2, np.float64)

    ag = agent64 @ Wag64 + bag64
    q_agent, k_agent = ag[:, :D], ag[:, D:]
    W_q, W_k, W_v = Wqkv64[:, :D], Wqkv64[:, D : 2 * D], Wqkv64[:, 2 * D :]
    b_q, b_v = bqkv64[:D], bqkv64[2 * D :]

    wk_f = W_k @ q_agent.T                      # [D, K]
    wq_f = W_q @ k_agent.T                      # [D, K]
    c2_f = (D ** -0.5) * (k_agent @ b_q)        # [K]
    Wbig = W_v @ Wf1 @ Wf2                      # [D, D]
    bbig = (b_v @ Wf1 + bf1) @ Wf2 + bf2        # [D], added on host

    # [D, D] -> [P, DS, D] with d = s*128 + p
    wcombo_b = np.ascontiguousarray(
        Wbig.reshape(DS, P, D).transpose(1, 0, 2)
    ).astype(_BF16)
    wkq8 = np.concatenate([wk_f, wq_f], axis=1).reshape(DS, P, 2 * K)
    wkq8 = np.ascontiguousarray(wkq8.transpose(1, 0, 2)).astype(_FP8)
    fc = np.ascontiguousarray(
        (c2_f - 2.772588722239781)[:, None]
    ).astype(_BF16)

    x32 = np.asarray(x, np.float32)
    # xe pack: [B, N, D] -> [B, P, NT, D], token = t*128 + p
    xeb = np.ascontiguousarray(
        x32.astype(_FP8).reshape(B, NT, P, D).transpose(0, 2, 1, 3)
    )
    # xT pack: [B, D, N] -> [B, P, DS, N], d = s*128 + p
    xTb = x32.transpose(0, 2, 1).reshape(B, DS, P, N)
    xTb = np.ascontiguousarray(xTb.transpose(0, 2, 1, 3)).astype(_FP8)

    in_maps = [
        {
            "xT": xTb[i],
            "xe": xeb[i],
            "wkq8": wkq8,
            "wcombo": wcombo_b,
            "fc": fc,
        }
        for i in range(B)
    ]

    return in_maps, x32, bbig.astype(np.float32)


def kernel(**inputs):
    from concourse.bass_utils import run_bass_kernel_spmd

    in_maps, x32, bbig = _prepare_in_maps(**inputs)
    nc = _get_nc()
    res_obj = run_bass_kernel_spmd(nc, in_maps, core_ids=list(range(B)))
    _CACHE["last_results"] = res_obj
    res = res_obj.results

    # ye [P, NT, D] -> [N, D] with token = t*128 + p
    ye = np.stack(
        [np.asarray(res[i]["ye"]).transpose(1, 0, 2).reshape(N, D) for i in range(B)]
    ).astype(np.float32)
    yden = np.stack(
        [np.asarray(res[i]["yden"]).transpose(1, 0).reshape(N) for i in range(B)]
    ).astype(np.float32)
    out = ye / yden[:, :, None] + bbig[None, None, :] + x32
    return out.astype(np.float32)
